# revision 1
# baseline (speedup 1.0000x reference)
"""GroupedQueryAttention TRN2 kernel v2 — all-bf16, pipelined, raw Bass.

Per core (8 cores = 4 batches x 2 head-groups): 16 q-heads (8 pairs),
4 kv-heads, full 1024-seq causal attention + out-projection partial.

Key structure vs v1:
  - every matmul bf16 (FWL weight loads pipeline behind streaming -> ~223ns/MM
    at N=512 instead of fp32r's ~400ns)
  - scores K=64 row-paired via tile_position (two heads concurrently)
  - causal trim on scores/PV N; exp batched over 4 PSUM banks per call,
    diagonal groups column-trimmed
  - softmax recip via ACT ln -> ACT exp(-x) -> one-hot bf16 broadcast matmul
  - RoPE: DVE copy, DMA partition-swap shuffle, 3 bf16 DVE ops
  - single pipelined pass; hand scheduling with a vector-clock race checker
"""
import numpy as np
import ml_dtypes
import concourse.bass as bass
import concourse.mybir as mybir
from concourse.bass_utils import run_bass_kernel_spmd

F32 = mybir.dt.float32
BF16 = mybir.dt.bfloat16
AF = mybir.ActivationFunctionType

B, S, HID = 4, 1024, 2048
NH, NKV, HD = 32, 8, 64
FP = 8      # q-head pairs per core
KT = 16     # k tiles over hidden
THETA = 10000.0
ROT = [4, 5, 6]   # PV/bcast psum bank rotation
DEBUG = False
CHECK = True
_CACHE = {}


class Sched:
    ENG = ("pe", "act", "dve", "ld", "gp")

    def __init__(self):
        self.prog = {e: [] for e in self.ENG}
        self.cnt = {e: 0 for e in self.ENG}
        self.cnt["st"] = 0
        self.waited = {e: {} for e in self.ENG}

    def wait(self, e, sem, val):
        if val is None or val <= 0:
            return
        if self.waited[e].get(sem, 0) >= val:
            return
        self.waited[e][sem] = val
        self.prog[e].append(("w", sem, val))

    def op(self, e, fn, reads=(), writes=(), inc=False, dma=False, sem=None):
        tgt = sem or e
        amt = 16 if dma else 1
        n = None
        if inc or dma:
            self.cnt[tgt] += amt
            n = self.cnt[tgt]
            self.prog[e].append(("o", fn, (tgt, amt), list(reads), list(writes)))
        else:
            self.prog[e].append(("o", fn, None, list(reads), list(writes)))
        return n


def check_races(s: Sched):
    """Vector-clock happens-before verification of the emitted program."""
    ops = {e: [] for e in s.ENG}
    for e in s.ENG:
        pend = []
        for item in s.prog[e]:
            if item[0] == "w":
                pend.append((item[1], item[2]))
            else:
                ops[e].append({"waits": pend, "inc": item[2],
                               "reads": item[3], "writes": item[4]})
                pend = []
    # sem event list: sem -> [(value_after, engine, op_idx)]
    sem_ev = {}
    for e in s.ENG:
        acc = {}
        for i, o in enumerate(ops[e]):
            if o["inc"]:
                tgt, amt = o["inc"]
                acc[tgt] = acc.get(tgt, 0) + amt
                sem_ev.setdefault(tgt, []).append((acc[tgt], e, i))
    eidx = {e: k for k, e in enumerate(s.ENG)}
    ptr = {e: 0 for e in s.ENG}
    cur = {e: [-1] * len(s.ENG) for e in s.ENG}
    vc = {e: [None] * len(ops[e]) for e in s.ENG}
    order = []
    progressed = True
    while progressed:
        progressed = False
        for e in s.ENG:
            while ptr[e] < len(ops[e]):
                o = ops[e][ptr[e]]
                joins = []
                ok = True
                for (sem, val) in o["waits"]:
                    j = None
                    for (v, en, i) in sem_ev.get(sem, []):
                        if v >= val:
                            j = (en, i)
                            break
                    if j is None:
                        raise RuntimeError(f"{e}: wait {sem}>={val} never satisfied")
                    en, i = j
                    if vc[en][i] is None:
                        ok = False
                        break
                    joins.append((en, i))
                if not ok:
                    break
                myvc = list(cur[e])
                for (en, i) in joins:
                    pv = vc[en][i]
                    for k in range(len(myvc)):
                        if pv[k] > myvc[k]:
                            myvc[k] = pv[k]
                    if i > myvc[eidx[en]]:
                        myvc[eidx[en]] = i
                myvc[eidx[e]] = ptr[e]
                vc[e][ptr[e]] = myvc
                cur[e] = myvc
                order.append((e, ptr[e]))
                ptr[e] += 1
                progressed = True
    for e in s.ENG:
        if ptr[e] < len(ops[e]):
            raise RuntimeError(f"deadlock: {e} stuck at op {ptr[e]}/{len(ops[e])} "
                               f"waits={ops[e][ptr[e]]['waits']}")
    # hazard check over the topological order
    last_w = {}
    readers = {}
    errs = []

    def ordered(a, b):
        if a[0] == b[0]:
            return a[1] <= b[1]
        return vc[b[0]][b[1]][eidx[a[0]]] >= a[1]

    for (e, i) in order:
        o = ops[e][i]
        me = (e, i)
        for r in o["reads"]:
            w = last_w.get(r)
            if w is not None and not ordered(w, me):
                errs.append(f"RAW race on {r}: write {w} vs read {me}")
            readers.setdefault(r, []).append(me)
        for wkey in o["writes"]:
            w = last_w.get(wkey)
            if w is not None and not ordered(w, me):
                errs.append(f"WAW race on {wkey}: {w} vs {me}")
            for rd in readers.get(wkey, []):
                if rd != me and not ordered(rd, me):
                    errs.append(f"WAR race on {wkey}: read {rd} vs write {me}")
            last_w[wkey] = me
            readers[wkey] = []
    if errs:
        raise RuntimeError("RACES:\n" + "\n".join(errs[:40]))


def _build_nc():
    nc = bass.Bass(dynamic_dma_scratch_size=32768)

    ht_d = nc.declare_dram_parameter("ht", [128, 16, 1024], BF16, isOutput=False)
    wq_d = nc.declare_dram_parameter("wq", [128, 8, 16, 128], BF16, isOutput=False)
    wk_d = nc.declare_dram_parameter("wk", [128, 2, 16, 128], BF16, isOutput=False)
    wv_d = nc.declare_dram_parameter("wv", [128, 16, 256], BF16, isOutput=False)
    wo_d = nc.declare_dram_parameter("wo", [128, 4, 8, 512], BF16, isOutput=False)
    cos_d = nc.declare_dram_parameter("cosd", [128, 1024], BF16, isOutput=False)
    sin_d = nc.declare_dram_parameter("sinr", [128, 1024], BF16, isOutput=False)
    mk_d = nc.declare_dram_parameter("masks", [128, 128], BF16, isOutput=False)
    e01_d = nc.declare_dram_parameter("e01", [128, 128], BF16, isOutput=False)
    out_d = nc.declare_dram_parameter("out", [1024, 2048], BF16, isOutput=True)
    dbg_d = None
    if DEBUG:
        dbg_d = nc.declare_dram_parameter("dbg", [128, 8, 3, 1024], BF16, isOutput=True)

    off = (nc.sbuf_base + 63) & ~63   # skip bass const-AP region
    def sb(name, shape, dt):
        nonlocal off
        h = nc.alloc_sbuf_tensor_at(name, shape, dt, offset=off)
        n = 1
        for x in shape[1:]:
            n *= x
        off += n * mybir.dt.size(dt)
        off = (off + 31) & ~31
        return h

    HT = sb("HT", [128, 16, 1024], BF16)
    QT = sb("QT", [128, 8, 1024], BF16)
    KTr = sb("KTr", [128, 4, 1024], BF16)
    VA = sb("VA", [128, 8, 4, 65], BF16)
    OT = sb("OT", [128, 8, 1024], BF16)
    wq = sb("wq", [128, 4, 16, 128], BF16)
    wk = sb("wk", [128, 2, 16, 128], BF16)
    wv = sb("wv", [128, 16, 256], BF16)
    wo = sb("wo", [128, 4, 8, 512], BF16)
    cosd = sb("cosd", [128, 1024], BF16)
    sinr = sb("sinr", [128, 1024], BF16)
    qb = sb("qb", [128, 2, 512], BF16)
    qs = sb("qs", [128, 2, 512], BF16)
    tmp = sb("tmp", [128, 2, 512], BF16)
    kro = sb("kro", [128, 2, 512], BF16)
    exS = sb("exS", [128, 8, 512], BF16)
    masks = sb("masks", [128, 128], BF16)
    E01 = sb("E01", [128, 128], BF16)
    RL = sb("RL", [128, 2, 1024], F32)
    RZ = sb("RZ", [128, 2, 512], BF16)
    rb = sb("rb", [128, 2, 512], BF16)
    stg = sb("stg", [128, 4, 512], BF16)
    wz = sb("wz", [128, 512], BF16)

    PS = nc.alloc_psum_tensor("PS", [128, 8, 512], F32)

    s = Sched()
    W, O = s.wait, s.op

    def mm(out, lhsT, rhs, start, stop, tp=None):
        def fn(out=out, lhsT=lhsT, rhs=rhs, start=start, stop=stop, tp=tp):
            return nc.tensor.matmul(out, lhsT, rhs, start=start, stop=stop,
                                    skip_group_check=True, tile_position=tp)
        return fn

    # ---------------- init memsets (dve) ----------------
    O("dve", lambda: nc.vector.memset(wz[:], 0.0), writes=[("wz",)], inc=True)
    O("dve", lambda: nc.vector.memset(RZ[:], 0.0), writes=[("rzbg",)], inc=True)
    n_vaones = O("dve", lambda: nc.vector.memset(VA[:, :, :, 64:65], 1.0),
                 writes=[("vaones",)], inc=True)
    wz_done = n_vaones

    # ---------------- loads (sync queue) ----------------
    ld = {}
    def load(name, dst, src, key=None):
        ld[name] = O("ld", (lambda dst=dst, src=src:
                            nc.sync.dma_start(out=dst, in_=src)),
                     writes=[key or (name,)], dma=True)

    load("wv", wv[:], wv_d[:])
    for c in range(4):
        load(f"ht{c}", HT[:, 4*c:4*c+4, :], ht_d[:, 4*c:4*c+4, :], key=("ht", c))
    load("wk", wk[:], wk_d[:])
    load("cos", cosd[:], cos_d[:])
    load("sin", sinr[:], sin_d[:])
    load("wq0", wq[:, 0], wq_d[:, 0], key=("wq", 0))
    load("wq1", wq[:, 1], wq_d[:, 1], key=("wq", 1))
    load("wq2", wq[:, 2], wq_d[:, 2], key=("wq", 2))
    load("wq3", wq[:, 3], wq_d[:, 3], key=("wq", 3))
    load("masks", masks[:], mk_d[:])
    load("e01", E01[:], e01_d[:])

    # ---------------- warmup (pe) ----------------
    W("pe", "dve", wz_done)
    for i in range(20):
        O("pe", mm(PS[:, 7, :], wz[:, 0:128], wz[:], True, True),
          reads=[("wz",)], writes=[("ps", 7)])
    warm_done = O("pe", mm(PS[:, 7, :], wz[:, 0:128], wz[:], True, True),
                  reads=[("wz",)], writes=[("ps", 7)], inc=True)

    # ---------------- V projection ----------------
    vstop = {}
    for k in range(KT):
        W("pe", "ld", ld[f"ht{k//4}"])
        W("pe", "ld", ld["wv"])
        for rt in range(8):
            n = O("pe", mm(PS[:, rt, 0:256], HT[:, k, 128*rt:128*rt+128],
                           wv[:, k, :], k == 0, k == KT-1),
                  reads=[("ht", k // 4), ("wv",)], writes=[("ps", rt)],
                  inc=(k == KT-1))
            if k == KT-1:
                vstop[rt] = n

    va_done = {}
    for rt in range(8):
        W("dve", "pe", vstop[rt])
        for kv in range(4):
            n = O("dve", (lambda rt=rt, kv=kv: nc.vector.tensor_copy(
                    out=VA[:, rt, kv, 0:64], in_=PS[:, rt, 64*kv:64*kv+64])),
                  reads=[("ps", rt)], writes=[("va", rt, kv)], inc=True)
        va_done[rt] = n

    bank_free = {b: ("dve", va_done[b]) for b in (4, 5, 6)}

    # ---------------- rope helper ----------------
    shufn = {}     # tile -> ld count after its 4 shuffle dmas
    ropedone = {}  # tile key -> dve count of final add
    qcopyn = {}    # tile key -> dve count of psum->qb copy
    tile_seq = []  # emission order of rope tiles

    def rope_copy(key, bank):
        slot = len(tile_seq) % 2
        tile_seq.append(key)
        W("dve", "pe", key[3])  # proj stop count passed in key[3]
        qcopyn[key[:3]] = O("dve", (lambda bank=bank, slot=slot:
                nc.vector.tensor_copy(out=qb[:, slot, :], in_=PS[:, bank, :])),
              reads=[("ps", bank)], writes=[("qb", slot)], inc=True)
        return slot

    def rope_shuffle(key3, slot, queues=("gp", "gp", "gp", "gp")):
        # partition swap 32-blocks: qs[i^32] = qb[i]
        prev = tile_seq[-3] if len(tile_seq) >= 3 else None
        for qe in set(queues):
            W(qe, "dve", qcopyn[key3])
            # WAR on qs slot: previous cosmul output consumed by previous add
            if prev is not None:
                W(qe, "dve", ropedone[prev[:3]])
        ends = {}
        for bi, (qe, (d0, s0)) in enumerate(
                zip(queues, ((32, 0), (0, 32), (96, 64), (64, 96)))):
            fn = (lambda d0=d0, s0=s0, slot=slot, qe=qe:
                  (nc.sync if qe == "ld" else nc.gpsimd).dma_start(
                      out=qs[d0:d0+32, slot, :], in_=qb[s0:s0+32, slot, :]))
            ends[qe] = O(qe, fn, reads=[("qb", slot)],
                         writes=[("qs", slot, bi)], dma=True)
        shufn[key3] = list(ends.items())

    def rope_muls(key3, slot, dst_ap, dst_key, kro_war=None):
        for (qe, nn) in shufn[key3]:
            W("dve", qe, nn)
        W("dve", "ld", ld["sin"])
        rwin = key3[2] * 512  # r*512 col window
        O("dve", (lambda slot=slot, rwin=rwin: nc.vector.tensor_mul(
                out=tmp[:, slot, :], in0=qs[:, slot, :],
                in1=sinr[:, rwin:rwin+512])),
          reads=[("qs", slot, 0), ("qs", slot, 1), ("qs", slot, 2),
                 ("qs", slot, 3), ("sin",)], writes=[("tmp", slot)], inc=True)
        W("dve", "ld", ld["cos"])
        O("dve", (lambda slot=slot, rwin=rwin: nc.vector.tensor_mul(
                out=qs[:, slot, :], in0=qb[:, slot, :],
                in1=cosd[:, rwin:rwin+512])),
          reads=[("qb", slot), ("cos",)],
          writes=[("qs", slot, 0), ("qs", slot, 1), ("qs", slot, 2),
                  ("qs", slot, 3)], inc=True)
        if kro_war is not None:
            W("dve", "gp", kro_war)
        ropedone[key3] = O("dve", (lambda slot=slot, dst_ap=dst_ap:
                nc.vector.tensor_add(out=dst_ap, in0=tmp[:, slot, :],
                                     in1=qs[:, slot, :])),
              reads=[("tmp", slot), ("qs", slot, 0), ("qs", slot, 1),
                     ("qs", slot, 2), ("qs", slot, 3)],
              writes=[dst_key], inc=True)

    # ---------------- interleaved K + head-Q projections ----------------
    kstop = {}
    krepn = {}
    kt_idx = [0]

    def emit_kproj(kf, r):
        t = 2 * kf + r
        bank = 4 + t % 2
        W("pe", "ld", ld["wk"])
        for c in range(4):
            W("pe", "ld", ld[f"ht{c}"])
        W("pe", "dve", va_done[bank])
        if t >= 2:
            W("pe", "dve", qcopyn[("k", kf - 1, r)])
        n = None
        for k in range(KT):
            n = O("pe", mm(PS[:, bank, :], wk[:, kf, k, :],
                           HT[:, k, 512*r:512*r+512], k == 0, k == KT-1),
                  reads=[("wk",), ("ht", k // 4)], writes=[("ps", bank)],
                  inc=(k == KT-1))
        kstop[(kf, r)] = n
        key = ("k", kf, r, n)
        slot = rope_copy(key, bank)
        rope_shuffle(key[:3], slot, queues=("ld", "gp", "ld", "gp"))
        kslot = t % 2
        war = krepn.get(t - 2)
        rope_muls(key[:3], slot, kro[:, kslot, :], ("kro", kslot), kro_war=war)
        # replicate rows 0:64 -> KTr[kv=2kf] both halves, 64:128 -> kv=2kf+1
        W("gp", "dve", ropedone[key[:3]])
        n = None
        for (kv, srow, drow) in ((2*kf, 0, 0), (2*kf, 0, 64),
                                 (2*kf+1, 64, 0), (2*kf+1, 64, 64)):
            n = O("gp", (lambda kv=kv, srow=srow, drow=drow, r=r, kslot=kslot:
                    nc.gpsimd.dma_start(
                        out=KTr[drow:drow+64, kv, 512*r:512*r+512],
                        in_=kro[srow:srow+64, kslot, :])),
                  reads=[("kro", kslot)], writes=[("kt", kv, r)], dma=True)
        krepn[t] = n


    # ---------------- Q projection tile ----------------
    qstop = {}
    proj_prev = {6: None, 7: None}
    proj_bank = {}

    def emit_proj(f, r, bank=7):
        W("pe", "ld", ld_wq[f])
        W("pe", "dve", va_done[bank])
        if proj_prev[bank] is not None:
            W("pe", "dve", qcopyn[proj_prev[bank]])
        n = None
        for k in range(KT):
            n = O("pe", mm(PS[:, bank, :], wq[:, f % 4, k, :],
                           HT[:, k, 512*r:512*r+512], k == 0, k == KT-1),
                  reads=[("wq", f % 4), ("ht", k // 4)], writes=[("ps", bank)],
                  inc=(k == KT-1))
        qstop[(f, r)] = n
        key = ("q", f, r, n)
        proj_prev[bank] = key[:3]
        proj_bank[key[:3]] = bank
        return key

    rope_slot = {}

    def emit_proj_rope(key, head=False):
        f, r = key[1], key[2]
        slot = rope_copy(key, proj_bank[key[:3]])
        rope_shuffle(key[:3], slot,
                     queues=("ld", "gp", "ld", "gp") if head else
                            ("gp", "gp", "gp", "gp"))
        if head:
            rope_muls(key[:3], slot, QT[:, f, 512*r:512*r+512], ("qt", f, r))
        else:
            rope_slot[key[:3]] = slot

    def emit_rope_muls(key3):
        f, r = key3[1], key3[2]
        rope_muls(key3, rope_slot[key3], QT[:, f, 512*r:512*r+512],
                  ("qt", f, r))

    ld_wq = {0: ld["wq0"], 1: ld["wq1"], 2: ld["wq2"], 3: ld["wq3"]}

    def load_wq(f):
        W("ld", "pe", qstop[(f - 4, 1)])
        ld_wq[f] = O("ld", (lambda f=f: nc.sync.dma_start(
                out=wq[:, f % 4], in_=wq_d[:, f])),
              writes=[("wq", f % 4)], dma=True)

    emit_kproj(0, 0)
    emit_proj_rope(emit_proj(0, 0, bank=6), head=True)
    emit_kproj(0, 1)
    emit_proj_rope(emit_proj(0, 1, bank=7), head=True)
    emit_kproj(1, 0)
    emit_proj_rope(emit_proj(1, 0, bank=6), head=True)
    emit_kproj(1, 1)
    emit_proj_rope(emit_proj(1, 1, bank=7), head=True)
    bank_free[4] = ("dve", qcopyn[("k", 1, 0)])
    bank_free[5] = ("dve", qcopyn[("k", 1, 1)])
    bank_free[6] = ("dve", qcopyn[("q", 1, 0)])
    load("wo0", wo[:, 0], wo_d[:, 0], key=("wo", 0))
    load("wo1", wo[:, 1], wo_d[:, 1], key=("wo", 1))

    # ---------------- stage B ----------------
    gi_ctr = [0]
    exp_done = {}          # gi -> act count
    pvgrp = {}             # gi -> pe count of last PV mm of that group
    sb_free = [("dve", va_done[b]) for b in range(4)]  # score banks 0-3
    maskn = {}
    norm_done = {}
    rbevac = {}
    bcastn = {}
    recipn = {}
    exp_reads_banks = [None]  # act count of last exp (stage C gating)
    pending_den = []
    pending_rope = []
    pending_chain = []

    def emit_unit(f, Q):
        u = 2 * f + Q
        kv = f // 2
        nct = 4 * Q + 4
        groups = [(c, c + 1) for c in range(0, nct, 2)]
        bA, bB = ROT[(2 * u) % 3], ROT[(2 * u + 1) % 3]
        bc = ROT[(2 * u + 2) % 3]

        def sgroup(gl):
            gi = gi_ctr[0]
            c0 = groups[gl][0]
            # score banks 0-3 freed by exp of previous group
            if gi > 0:
                W("pe", "act", exp_done[gi - 1])
            else:
                for (sem, val) in sb_free:
                    W("pe", sem, val)
            W("pe", "dve", ropedone[("q", f, Q)])
            for ci in (c0, c0 + 1):
                rK = ci // 4
                W("pe", "gp", krepn[{(0, 0): 0, (0, 1): 1, (1, 0): 2, (1, 1): 3}[(kv // 2, rK)]])
                tc = 128 * (ci - 4 * Q) if ci >= 4 * Q else 0
                for m in (0, 1):
                    sbk = 2 * (ci - c0) + m
                    n = O("pe", mm(PS[:, sbk, tc:512],
                                   KTr[64*m:64*m+64, kv, 128*ci:128*ci+128],
                                   QT[64*m:64*m+64, f, 512*Q+tc:512*Q+512],
                                   True, True, tp=(64 * m, 0)),
                          reads=[("kt", kv, rK), ("qt", f, Q)],
                          writes=[("ps", sbk)],
                          inc=(ci == c0 + 1 and m == 1))
            # exp on act
            W("act", "pe", n)
            if gi >= 2:
                W("act", "pe", pvgrp[gi - 2])
            slot0 = (gi % 2) * 4
            trim = 256 if (c0 >= 4 * Q and c0 - 4 * Q >= 2) else 0
            exp_done[gi] = O("act", (lambda slot0=slot0, trim=trim:
                    nc.scalar.activation(
                        out=exS[:, slot0:slot0+4, trim:512],
                        in_=PS[:, 0:4, trim:512], func=AF.Exp)),
                  reads=[("ps", 0), ("ps", 1), ("ps", 2), ("ps", 3)],
                  writes=[("exs", slot0 + i) for i in range(4)], inc=True)
            exp_reads_banks[0] = exp_done[gi]
            # masks on dve
            for ci in (c0, c0 + 1):
                if ci >= 4 * Q:
                    mc = 128 * (ci - 4 * Q)
                    for m in (0, 1):
                        slot = slot0 + 2 * (ci - c0) + m
                        W("dve", "act", exp_done[gi])
                        W("dve", "ld", ld["masks"])
                        maskn[(u, ci, m)] = O("dve",
                            (lambda slot=slot, mc=mc: nc.vector.tensor_mul(
                                out=exS[:, slot, mc:mc+128],
                                in0=exS[:, slot, mc:mc+128],
                                in1=masks[:])),
                            reads=[("exs", slot), ("masks",)],
                            writes=[("exs", slot)], inc=True)
            gi_ctr[0] += 1
            return gi

        def pvgroup(gl, gi):
            c0 = groups[gl][0]
            slot0 = (gi % 2) * 4
            W("pe", "act", exp_done[gi])
            for ci in (c0, c0 + 1):
                tc = 128 * (ci - 4 * Q) if ci >= 4 * Q else 0
                for m in (0, 1):
                    bk = bA if m == 0 else bB
                    slot = slot0 + 2 * (ci - c0) + m
                    if ci >= 4 * Q:
                        W("pe", "dve", maskn[(u, ci, m)])
                    if ci == 0:
                        W("pe", "dve", n_vaones)
                        fr = bank_free[bk]
                        W("pe", fr[0], fr[1])
                    n = O("pe", mm(PS[0:65, bk, tc:512], VA[:, ci, kv, 0:65],
                                   exS[:, slot, tc:512], ci == 0, ci == nct - 1),
                          reads=[("exs", slot), ("va", ci, kv), ("vaones",)],
                          writes=[("ps", bk)],
                          inc=(ci == c0 + 1 and m == 1))
            pvgrp[gi] = n
            return n

        gis = []
        gis.append(sgroup(0))
        proj_key = None
        if f + 2 < FP and Q == 0:
            proj_key = emit_proj(f + 2, 0)
        elif f + 2 < FP:
            proj_key = emit_proj(f + 2, 1)
            if f + 4 < FP:
                load_wq(f + 4)
        elif f == 6 and Q == 1:
            load("wo2", wo[:, 2], wo_d[:, 2], key=("wo", 2))
            load("wo3", wo[:, 3], wo_d[:, 3], key=("wo", 3))
        gis.append(sgroup(1))
        # flush previous unit's deferred den tail (bcast on PE, evac+norms on DVE)
        for fn in pending_den:
            fn()
        pending_den.clear()
        # rope muls for the proj tile shuffled last unit; then this unit's
        # copy+shuffle
        for k3 in pending_rope:
            emit_rope_muls(k3)
        pending_rope.clear()
        if proj_key is not None:
            emit_proj_rope(proj_key)
            pending_rope.append(proj_key[:3])
        for gl in range(1, len(groups)):
            if gl >= 2:
                gis.append(sgroup(gl))
            pvstop_n = pvgroup(gl - 1, gis[gl - 1])
            if f == 7 and Q == 1 and gl == 3:
                exp_reads_banks[0] = exp_done[gis[-1]]
                for t in (0, 1, 2, 3, 8, 9, 10, 11):
                    emit_sc_tile(t)
        pvstop_n = pvgroup(len(groups) - 1, gis[-1])

        # den: ln -> exp(-x) -> bcast -> evac -> norms
        W("act", "pe", pvstop_n)
        slot = u % 2
        O("act", (lambda bA=bA, slot=slot: nc.scalar.activation(
                out=RL[0:1, slot, 0:512], in_=PS[64:65, bA, :],
                func=AF.Ln)),
          reads=[("ps", bA)], writes=[("rl", slot)], inc=True)
        O("act", (lambda bB=bB, slot=slot: nc.scalar.activation(
                out=RL[0:1, slot, 512:1024], in_=PS[64:65, bB, :],
                func=AF.Ln)),
          reads=[("ps", bB)], writes=[("rl", slot)], inc=True)
        if u >= 2:
            W("act", "pe", bcastn[u - 2])
        O("act", (lambda slot=slot: nc.scalar.activation(
                out=RZ[0:1, slot, :], in_=RL[0:1, slot, 0:512],
                func=AF.Exp, scale=-1.0)),
          reads=[("rl", slot)], writes=[("rz", slot)], inc=True)
        recipn[u] = O("act", (lambda slot=slot: nc.scalar.activation(
                out=RZ[32:33, slot, :], in_=RL[0:1, slot, 512:1024],
                func=AF.Exp, scale=-1.0)),
              reads=[("rl", slot)], writes=[("rz", slot)], inc=True)
        def den_tail(u=u, f=f, Q=Q, slot=slot, bA=bA, bB=bB, bc=bc,
                     pvstop_n=pvstop_n):
            W("pe", "act", recipn[u])
            W("pe", "ld", ld["e01"])
            fr = bank_free[bc]
            W("pe", fr[0], fr[1])
            bcastn[u] = O("pe", mm(PS[:, bc, :], E01[:], RZ[:, slot, :], True, True),
                          reads=[("rz", slot), ("e01",), ("rzbg",)],
                          writes=[("ps", bc)], inc=True)
            W("dve", "pe", bcastn[u])
            rbevac[u] = O("dve", (lambda bc=bc, slot=slot: nc.vector.tensor_copy(
                    out=rb[:, slot, :], in_=PS[:, bc, :])),
                  reads=[("ps", bc)], writes=[("rb", slot)], inc=True)
            bank_free[bc] = ("dve", rbevac[u])
            W("dve", "pe", pvstop_n)
            n = O("dve", (lambda bA=bA, slot=slot, f=f, Q=Q: nc.vector.tensor_mul(
                    out=OT[0:64, f, 512*Q:512*Q+512], in0=PS[0:64, bA, :],
                    in1=rb[0:64, slot, :])),
                  reads=[("ps", bA), ("rb", slot)], writes=[("ot", f, Q)], inc=True)
            bank_free[bA] = ("dve", n)
            n = O("dve", (lambda bB=bB, slot=slot, f=f, Q=Q: nc.vector.tensor_mul(
                    out=OT[64:128, f, 512*Q:512*Q+512], in0=PS[0:64, bB, :],
                    in1=rb[64:128, slot, :])),
                  reads=[("ps", bB), ("rb", slot)], writes=[("ot", f, Q)], inc=True)
            bank_free[bB] = ("dve", n)
            norm_done[u] = n
        pending_den.append(den_tail)

    sc_done = set()
    sc_state = {"i": 0}
    sc_bank_last = {}
    sc_slot_last = {}

    def emit_sc_tile(t):
        cs, rt = t // 8, t % 8
        i = sc_state["i"]
        sc_state["i"] += 1
        bk = i % 4
        slot = i % 4
        if bk in sc_bank_last:
            W("pe", "act", sc_bank_last[bk])
        else:
            W("pe", "act", exp_reads_banks[0])
        W("pe", "dve", norm_done[2 * FP - 2] if rt < 4
          else norm_done[2 * FP - 1])
        W("pe", "ld", ld[f"wo{cs}"])
        n = None
        for fi in range(8):
            n = O("pe", mm(PS[:, bk, :], OT[:, fi, 128*rt:128*rt+128],
                           wo[:, cs, fi, :], fi == 0, fi == 7),
                  reads=[("ot", fi, rt // 4), ("wo", cs)], writes=[("ps", bk)],
                  inc=(fi == 7))
        W("act", "pe", n)
        if slot in sc_slot_last:
            W("act", "st", sc_slot_last[slot])
        ev = O("act", (lambda bk=bk, slot=slot: nc.scalar.copy(
                out=stg[:, slot, :], in_=PS[:, bk, :])),
              reads=[("ps", bk)], writes=[("stg", slot)], inc=True)
        sc_bank_last[bk] = ev
        stn = O("act", (lambda cs=cs, rt=rt, slot=slot: nc.scalar.dma_start(
                out=out_d[128*rt:128*rt+128, 512*cs:512*cs+512],
                in_=stg[:, slot, :])),
          reads=[("stg", slot)], writes=[("out", t)], dma=True, sem="st")
        sc_slot_last[slot] = stn
        sc_done.add(t)

    for f in range(FP):
        emit_unit(f, 0)
        emit_unit(f, 1)
    for fn in pending_den:
        fn()
    pending_den.clear()
    for k3 in pending_rope:
        emit_rope_muls(k3)
    pending_rope.clear()

    # ---------------- stage C ----------------
    for t in [i for i in range(32) if i not in sc_done]:
        emit_sc_tile(t)

    if DEBUG:
        W("act", "st", 16 * 32)
        W("act", "dve", norm_done[2 * FP - 1])
        for f in range(8):
            O("act", (lambda f=f: nc.scalar.dma_start(
                    out=dbg_d[:, f, 0, :], in_=QT[:, f, :])),
              dma=True, sem="st")
        for kv in range(4):
            O("act", (lambda kv=kv: nc.scalar.dma_start(
                    out=dbg_d[:, kv, 1, :], in_=KTr[:, kv, :])),
              dma=True, sem="st")
        for f in range(8):
            O("act", (lambda f=f: nc.scalar.dma_start(
                    out=dbg_d[:, f, 2, :], in_=OT[:, f, :])),
              dma=True, sem="st")

    if CHECK:
        check_races(s)

    # ---------------- emit ----------------
    with (
        nc.Block() as block,
        nc.semaphore("s_pe") as s_pe,
        nc.semaphore("s_act") as s_act,
        nc.semaphore("s_dve") as s_dve,
        nc.semaphore("s_ld") as s_ld,
        nc.semaphore("s_gp") as s_gp,
        nc.semaphore("s_st") as s_st,
    ):
        sems = {"pe": s_pe, "act": s_act, "dve": s_dve, "ld": s_ld,
                "gp": s_gp, "st": s_st}

        def run(eng, lst):
            for item in lst:
                if item[0] == "w":
                    eng.wait_ge(sems[item[1]], item[2])
                else:
                    inst = item[1]()
                    if item[2] is not None:
                        inst.then_inc(sems[item[2][0]], item[2][1])

        @block.tensor
        def _(pe):
            run(pe, s.prog["pe"])

        @block.scalar
        def _(act):
            run(act, s.prog["act"])

        @block.vector
        def _(dve):
            run(dve, s.prog["dve"])

        @block.sync
        def _(sync):
            run(sync, s.prog["ld"])

        @block.gpsimd
        def _(gp):
            run(gp, s.prog["gp"])

    return nc


def _host_prep(hidden_states, position_ids, Wq, Wk, Wv, Wo):
    bf = ml_dtypes.bfloat16
    pos = position_ids.astype(np.float32)
    inv = 1.0 / (THETA ** (np.arange(0, HD, 2, dtype=np.float32) / HD))
    ang = pos[:, None] * inv[None, :]
    emb = np.concatenate([ang, ang], axis=1)          # [S, 64]
    cos_t = np.cos(emb).T.astype(np.float32)          # [64, S]
    sin_t = np.sin(emb).T.astype(np.float32)
    cosd = np.concatenate([cos_t, cos_t], axis=0).astype(bf)
    sgn = np.where(np.arange(HD) < HD // 2, -1.0, 1.0).astype(np.float32)
    sinr = np.concatenate([sin_t * sgn[:, None]] * 2, axis=0).astype(bf)

    kp = np.arange(128)[:, None]
    lq = np.arange(128)[None, :]
    masks = (lq >= kp).astype(np.float32).astype(bf)
    e01 = np.zeros((128, 128), dtype=np.float32)
    e01[0, 0:64] = 1.0
    e01[32, 64:128] = 1.0
    e01 = e01.astype(bf)

    scale = np.float32(HD ** -0.5)
    in_maps = []
    for cid in range(8):
        b, hg = cid // 2, cid % 2
        ht = np.ascontiguousarray(
            hidden_states[b].T.reshape(16, 128, 1024).transpose(1, 0, 2)).astype(bf)
        wq_s = (Wq[:, hg*1024:(hg+1)*1024] * scale)
        wqr = np.ascontiguousarray(
            wq_s.reshape(16, 128, 8, 128).transpose(1, 2, 0, 3)).astype(bf)
        wk_s = Wk[:, hg*256:(hg+1)*256]
        wkr = np.ascontiguousarray(
            wk_s.reshape(16, 128, 2, 128).transpose(1, 2, 0, 3)).astype(bf)
        wv_s = Wv[:, hg*256:(hg+1)*256]
        wvr = np.ascontiguousarray(
            wv_s.reshape(16, 128, 256).transpose(1, 0, 2)).astype(bf)
        wo_s = Wo[hg*1024:(hg+1)*1024, :]
        wor = np.ascontiguousarray(
            wo_s.reshape(8, 128, 4, 512).transpose(1, 2, 0, 3)).astype(bf)
        in_maps.append({"ht": ht, "wq": wqr, "wk": wkr, "wv": wvr, "wo": wor,
                        "cosd": cosd, "sinr": sinr, "masks": masks, "e01": e01})
    return in_maps


def kernel(hidden_states, attention_mask, position_ids, Wq, Wk, Wv, Wo,
           _trace=False, _trace_kwargs=None):
    key = ("nc", DEBUG)
    if key not in _CACHE:
        _CACHE[key] = _build_nc()
    nc = _CACHE[key]
    in_maps = _host_prep(np.asarray(hidden_states), np.asarray(position_ids),
                         np.asarray(Wq), np.asarray(Wk), np.asarray(Wv),
                         np.asarray(Wo))
    kw = {}
    if _trace:
        kw = {"trace": True}
        if _trace_kwargs:
            kw.update(_trace_kwargs)
    res = run_bass_kernel_spmd(nc, in_maps, list(range(8)), **kw)
    full = np.empty((B, S, HID), dtype=np.float32)
    for b in range(B):
        full[b] = (res.results[2*b]["out"].astype(np.float32)
                   + res.results[2*b+1]["out"].astype(np.float32))
    kernel._last_result = res
    return full

